# revision 7
# baseline (speedup 1.0000x reference)
"""MoE grouped-GEMM (SiLU-gated FFN) kernel for 8 Trainium2 NeuronCores.

Strategy: expert-parallel along the intermediate dim with EXACT-token
slots (no 128-padding of token counts).  Each program slot is either
  - a QB=4 "pair" slot: expert A's four 4-block i-ranges on cores 0-3,
    expert B's on cores 4-7, slot width = max(nA, nB) tokens; or
  - a QB=2 "single" slot: one expert's eight 2-block i-ranges, one per
    core, slot width = exactly that expert's token count.
The pair/single split is chosen per batch_sizes by a cost model
max(tensor_us, dma_us): pairing halves x/out traffic for big experts at
the price of max() padding, singles are exact.  All matmul free dims use
the exact slot width, and the down projection is TRANSPOSED
(dn[128 h, tokens] = w2_blk.T @ gated) so phase-2 cost also scales with
exact tokens.  Tokens are routed host-side (free); per-i-range partial
down sums are combined host-side (free reduce).

On-core program (SPMD, identical widths on all 8 cores):
  per slot: phase 1 per (block, col-chunk): up/gt [128, cw] = w.T @ x
  accumulated over 8 H-chunks; SiLU (ScalarE); mul + bf16 cast (VectorE).
  phase 2 per (col-chunk, h): dnT [128, cw] accumulated over the slot's
  QB i-blocks with w2 blocks as stationary, copied bf16 to the output
  buffer (VectorE), stored whole-slot via SWDGE (last slot on the idle
  SP HW queue).
All matmuls bf16 with fp32 PSUM accumulation.

Performance notes: ~7us of fixed framework preamble precedes the kernel;
input DMA saturates the ~358GB/s per-core HBM read port, so weight bytes
(12.6MB/core, each byte loaded once) set the floor.  Loads are issued
greedily onto the two HW DGE queues (SP + ACT) balanced by bytes in
compute-consumption order.  A short dummy-matmul bridge keeps the PE busy
from the first post-preamble cycle so the HAM clock gate (1.2 -> 2.4GHz,
~5us sustained busy) opens during the initial x/w stream, and a dummy
SiLU activation pre-loads the scalar-engine act table off the critical
path.
"""

import os
import sys
from contextlib import ExitStack

import numpy as np

for _p in ("/opt/trn_rl_repo", "/root/.axon_site/_ro/trn_rl_repo"):
    if os.path.isdir(_p) and _p not in sys.path:
        sys.path.append(_p)

import ml_dtypes  # noqa: E402
import concourse.bass as bass  # noqa: E402
import concourse.mybir as mybir  # noqa: E402
import concourse.tile as tile  # noqa: E402
from concourse import bacc  # noqa: E402
from concourse.bass_utils import run_bass_kernel_spmd  # noqa: E402

BF16 = mybir.dt.bfloat16
F32 = mybir.dt.float32
BF16_NP = ml_dtypes.bfloat16

E, T, H, I = 8, 2048, 1024, 2048
NCORES = 8
TILE = 128
NB = I // TILE  # 16 i-blocks per expert
HC = H // TILE  # 8 h-chunks
WBLK = TILE * HC * TILE * 2  # bytes of one [H,128] weight block in bf16
CHUNK = 512  # max matmul free dim / PSUM bank cols (fp32)


def _ceil32(w):
    return -(-w // 32) * 32


def _chunks(w):
    """Split width w into ceil(w/CHUNK) near-equal col chunks (>=1 col)."""
    n = max(1, -(-w // CHUNK))
    base, rem = divmod(w, n)
    out = []
    c0 = 0
    for i in range(n):
        cw = base + (1 if i < rem else 0)
        out.append((c0, cw))
        c0 += cw
    return [(c0, cw) for c0, cw in out if cw > 0]


def _plan(bs):
    """Choose slots: list of (qb, experts) where experts is (a,) or (a, b).

    Pair slots put expert a on cores 0-3 (4-block jobs) and b on cores
    4-7; single slots give each core one 2-block job of the expert.
    """
    order_e = sorted(range(E), key=lambda e: (-int(bs[e]), e))
    real = [e for e in order_e if int(bs[e]) > 0]
    best = None
    for k in range(len(real) // 2 + 1):
        slots = [(4, (real[2 * i], real[2 * i + 1])) for i in range(k)]
        slots += [(2, (e,)) for e in real[2 * k:]]
        t_cyc = 0
        d_bytes = 0.0
        for qb, exps in slots:
            w = max(int(bs[e]) for e in exps)
            t_cyc += 24 * qb * w
            d_bytes += 3 * qb * WBLK + _ceil32(w) * H * 2
        cost = max(t_cyc / 2400.0, d_bytes / 358e3)
        if best is None or cost < best[0]:
            best = (cost, slots)
    slots = best[1]
    slots.sort(key=lambda s: -max(int(bs[e]) for e in s[1]))
    # ramp: lead with a smallish single slot (fast first loads) when the
    # biggest slot would otherwise stall the PE waiting for a large x
    if slots and max(int(bs[e]) for e in slots[0][1]) > 320:
        starters = [s for s in slots
                    if s[0] == 2 and 96 <= int(bs[s[1][0]]) <= 320]
        if starters:
            st = starters[0]
            slots.remove(st)
            slots.insert(0, st)
    return slots


def _build(key):
    """Build the SPMD Bass program. key = tuple of (qb, W, Wx) per slot."""
    nslot = len(key)
    XC = sum(HC * wx for _, _, wx in key)
    OC = sum(HC * w for _, w, _ in key)
    TOTB = sum(qb for qb, _, _ in key)
    WxM = max(wx for _, _, wx in key)  # tag-max sizes: pools are allocated
    WM = max(w for _, w, _ in key)     # at max and sliced per slot
    GM = {qb: max((w for q, w, _ in key if q == qb), default=0)
          for qb in (2, 4)}

    nc = bacc.Bacc("TRN2", target_bir_lowering=False, debug=False,
                   num_devices=NCORES)
    xt = nc.dram_tensor("xt", [TILE, XC], BF16, kind="ExternalInput").ap()
    w1 = nc.dram_tensor("w1", [TILE, TOTB * HC * TILE], BF16,
                        kind="ExternalInput").ap()
    w3 = nc.dram_tensor("w3", [TILE, TOTB * HC * TILE], BF16,
                        kind="ExternalInput").ap()
    w2 = nc.dram_tensor("w2", [TILE, TOTB * HC * TILE], BF16,
                        kind="ExternalInput").ap()
    out = nc.dram_tensor("out", [TILE, OC], BF16, kind="ExternalOutput").ap()

    with tile.TileContext(nc) as tc, ExitStack() as ctx:
        xpool = ctx.enter_context(tc.tile_pool(name="x", bufs=3))
        wq4 = ctx.enter_context(tc.tile_pool(name="wq4", bufs=2))
        wq2 = ctx.enter_context(tc.tile_pool(name="wq2", bufs=4))
        gpool = ctx.enter_context(tc.tile_pool(name="gated", bufs=2))
        apool = ctx.enter_context(tc.tile_pool(name="act", bufs=3))
        opool = ctx.enter_context(tc.tile_pool(name="osb", bufs=2))
        pup = ctx.enter_context(tc.tile_pool(name="pup", bufs=2, space="PSUM"))
        pgt = ctx.enter_context(tc.tile_pool(name="pgt", bufs=2, space="PSUM"))
        pdn = ctx.enter_context(tc.tile_pool(name="pdn", bufs=3, space="PSUM"))

        # PE warm-up bridge: keep the PE busy from the first post-preamble
        # cycle so the HAM clock gate opens while the first loads stream.
        wu_pool = ctx.enter_context(tc.tile_pool(name="wu", bufs=1))
        wu_l = wu_pool.tile([TILE, TILE], BF16, tag="wul")
        wu_r = wu_pool.tile([TILE, CHUNK], BF16, tag="wur")
        nc.vector.memset(wu_l[:], 0.0)
        nc.vector.memset(wu_r[:], 0.0)
        wu_ps = pdn.tile([TILE, CHUNK], F32, tag="dn")
        nc.tensor.matmul(wu_ps[:], wu_l[:], wu_r[:], start=True, stop=True)
        wu_act = apool.tile([TILE, CHUNK], F32, tag="act")
        for _ in range(3):
            nc.tensor.matmul(wu_ps[:], wu_l[:], wu_r[:], start=True, stop=True)
        for _ in range(6):
            wu_f = pdn.tile([TILE, CHUNK], F32, tag="dn")
            nc.tensor.matmul(wu_f[:, 0:TILE], wu_l[:], wu_r[:, 0:TILE],
                             start=True, stop=True)

        # greedy byte-balanced assignment onto the two HW DGE queues
        qeng = [nc.sync, nc.scalar]
        qbytes = [0, 0]
        act_warmed = [False]

        def issue(dst, src, nbytes):
            qi = 0 if qbytes[0] <= qbytes[1] else 1
            qeng[qi].dma_start(dst, src)
            qbytes[qi] += nbytes
            # act-table pre-load once, behind slot0's scalar-side issues
            if not act_warmed[0] and qbytes[1] > 0:
                nc.scalar.activation(wu_act[:, 0:1], wu_ps[:, 0:1],
                                     mybir.ActivationFunctionType.Silu)
                act_warmed[0] = True

        xoff = 0
        boff = 0
        ooff = 0
        BL = HC * TILE  # elems per block in w tensors
        for s, (qb, W, Wx) in enumerate(key):
            xlo = xpool.tile([TILE, (HC // 2) * WxM], BF16, tag="xlo")
            xhi = xpool.tile([TILE, (HC // 2) * WxM], BF16, tag="xhi")
            wp = wq4 if qb == 4 else wq2
            w1sb = wp.tile([TILE, qb * BL], BF16, tag=f"w1q{qb}")
            w3sb = wp.tile([TILE, qb * BL], BF16, tag=f"w3q{qb}")
            w2sb = wp.tile([TILE, qb * BL], BF16, tag=f"w2q{qb}")

            # loads, in compute-consumption order
            xb = (HC // 2) * Wx * 2
            issue(xlo[:, 0:(HC // 2) * Wx],
                  xt[:, xoff:xoff + (HC // 2) * Wx], xb)
            issue(w1sb[:, 0:BL], w1[:, (boff) * BL:(boff + 1) * BL], WBLK)
            issue(xhi[:, 0:(HC // 2) * Wx],
                  xt[:, xoff + (HC // 2) * Wx:xoff + HC * Wx], xb)
            issue(w3sb[:, 0:BL], w3[:, (boff) * BL:(boff + 1) * BL], WBLK)
            for b in range(1, qb):
                issue(w1sb[:, b * BL:(b + 1) * BL],
                      w1[:, (boff + b) * BL:(boff + b + 1) * BL], WBLK)
                issue(w3sb[:, b * BL:(b + 1) * BL],
                      w3[:, (boff + b) * BL:(boff + b + 1) * BL], WBLK)
            for b in range(qb):
                issue(w2sb[:, b * BL:(b + 1) * BL],
                      w2[:, (boff + b) * BL:(boff + b + 1) * BL], WBLK)

            def xs(h, c0, cw, Wx=Wx):
                t = xlo if h < HC // 2 else xhi
                return t[:, (h % (HC // 2)) * Wx + c0:
                         (h % (HC // 2)) * Wx + c0 + cw]

            chunks = _chunks(W)
            gated = gpool.tile([TILE, qb * GM[qb]], BF16, tag=f"g{qb}")
            for b in range(qb):
                for c0, cw in chunks:
                    up = pup.tile([TILE, CHUNK], F32, tag="up")
                    gt = pgt.tile([TILE, CHUNK], F32, tag="gt")
                    for h in range(HC):
                        nc.tensor.matmul(
                            up[:, 0:cw], w1sb[:, (b * HC + h) * TILE:
                                              (b * HC + h + 1) * TILE],
                            xs(h, c0, cw),
                            start=(h == 0), stop=(h == HC - 1))
                    for h in range(HC):
                        nc.tensor.matmul(
                            gt[:, 0:cw], w3sb[:, (b * HC + h) * TILE:
                                              (b * HC + h + 1) * TILE],
                            xs(h, c0, cw),
                            start=(h == 0), stop=(h == HC - 1))
                    act = apool.tile([TILE, CHUNK], F32, tag="act")
                    nc.scalar.activation(act[:, 0:cw], up[:, 0:cw],
                                         mybir.ActivationFunctionType.Silu)
                    nc.vector.tensor_mul(gated[:, b * W + c0:b * W + c0 + cw],
                                         act[:, 0:cw], gt[:, 0:cw])

            # transposed down projection: dnT[128 h, cols] over qb blocks
            osb = opool.tile([TILE, HC * WM], BF16, tag="osb")
            for c0, cw in chunks:
                for h in range(HC):
                    dn = pdn.tile([TILE, CHUNK], F32, tag="dn")
                    for b in range(qb):
                        nc.tensor.matmul(
                            dn[:, 0:cw], w2sb[:, (b * HC + h) * TILE:
                                              (b * HC + h + 1) * TILE],
                            gated[:, b * W + c0:b * W + c0 + cw],
                            start=(b == 0), stop=(b == qb - 1))
                    nc.vector.tensor_copy(osb[:, h * W + c0:h * W + c0 + cw],
                                          dn[:, 0:cw])
            if s == nslot - 1:
                # SP HW queue is idle by now; avoids the SWDGE drain tail
                nc.sync.dma_start(out[:, ooff:ooff + HC * W],
                                  osb[:, 0:HC * W])
            else:
                nc.gpsimd.dma_start(out[:, ooff:ooff + HC * W],
                                    osb[:, 0:HC * W])
            xoff += HC * Wx
            boff += qb
            ooff += HC * W
    nc.compile()
    return nc


def _ensure_ntff_hook():
    """Register the axon NTFF profile hook if the image's antenv lacks it."""
    import types
    try:
        from antenv.axon_hooks import get_axon_ntff_profile_hook  # noqa: F401
        return
    except ImportError:
        pass
    try:
        import antenv
        from trn_agent_boot.trn_boot import _ntff_profile_via_ctypes
        mod = types.ModuleType("antenv.axon_hooks")
        store = [None]
        mod.set_axon_ntff_profile_hook = lambda h: store.__setitem__(0, h)
        mod.get_axon_ntff_profile_hook = lambda: store[0]
        sys.modules["antenv.axon_hooks"] = mod
        antenv.axon_hooks = mod
        inner = _ntff_profile_via_ctypes("/opt/axon/libaxon_pjrt.so")

        import contextlib

        @contextlib.contextmanager
        def hook(output_dir, device_ids):
            import jax
            import jax.numpy as jnp
            jax.block_until_ready(jnp.add(jnp.ones(8), 1.0))
            with inner(output_dir, device_ids):
                yield

        mod.set_axon_ntff_profile_hook(hook if inner else None)
    except Exception as e:  # profiling is best-effort
        print(f"ntff hook registration failed: {e}", file=sys.stderr)


_CACHE = {}


def _get_program(key):
    if key not in _CACHE:
        _CACHE[key] = _build(key)
    return _CACHE[key]


def _run(hiddens, w1_weight, w2_weight, w3_weight, batch_sizes, trace=False):
    bs = np.asarray(batch_sizes, dtype=np.int64)
    starts = np.concatenate([[0], np.cumsum(bs)])
    slots = _plan(bs)
    key = tuple((qb, max(int(bs[e]) for e in exps),
                 _ceil32(max(int(bs[e]) for e in exps)))
                for qb, exps in slots)
    nc = _get_program(key)

    x = np.asarray(hiddens, dtype=np.float32)
    w1f = np.asarray(w1_weight)
    w2f = np.asarray(w2_weight)
    w3f = np.asarray(w3_weight)

    XC = sum(HC * wx for _, _, wx in key)
    OC = sum(HC * w for _, w, _ in key)
    TOTB = sum(qb for qb, _, _ in key)
    BL = HC * TILE

    def core_slot_job(c, s):
        qb, exps = slots[s]
        if qb == 4:
            e = exps[0] if c < 4 else exps[-1]
            c0 = 4 * (c % 4)
        else:
            e = exps[0]
            c0 = 2 * c
        return e, c0

    in_maps = []
    for c in range(NCORES):
        xt_np = np.zeros((TILE, XC), dtype=BF16_NP)
        w1_np = np.zeros((TILE, TOTB * BL), dtype=BF16_NP)
        w3_np = np.zeros((TILE, TOTB * BL), dtype=BF16_NP)
        w2_np = np.zeros((TILE, TOTB * BL), dtype=BF16_NP)
        xoff = 0
        boff = 0
        for s, (qb, W, Wx) in enumerate(key):
            e, c0 = core_slot_job(c, s)
            n_e = int(bs[e])
            if n_e > 0:
                xe = x[starts[e]:starts[e] + n_e]  # [n_e, H]
                blk = np.zeros((TILE, HC, Wx), dtype=BF16_NP)
                blk[:, :, :n_e] = (xe.T.reshape(HC, TILE, n_e)
                                   .transpose(1, 0, 2).astype(BF16_NP))
                xt_np[:, xoff:xoff + HC * Wx] = blk.reshape(TILE, HC * Wx)
            # w1/w3 lhsT blocks: [p(h_in_chunk), (b, h_chunk, i)]
            w1_np[:, boff * BL:(boff + qb) * BL] = (
                w1f[e].reshape(HC, TILE, NB, TILE)[:, :, c0:c0 + qb, :]
                .transpose(1, 2, 0, 3).astype(BF16_NP).reshape(TILE, qb * BL))
            w3_np[:, boff * BL:(boff + qb) * BL] = (
                w3f[e].reshape(HC, TILE, NB, TILE)[:, :, c0:c0 + qb, :]
                .transpose(1, 2, 0, 3).astype(BF16_NP).reshape(TILE, qb * BL))
            # w2 lhsT blocks: [p(i_in_block), (b, h_chunk, j)]
            w2_np[:, boff * BL:(boff + qb) * BL] = (
                w2f[e].reshape(NB, TILE, HC, TILE)[c0:c0 + qb]
                .transpose(1, 0, 2, 3).astype(BF16_NP).reshape(TILE, qb * BL))
            xoff += HC * Wx
            boff += qb
        in_maps.append({"xt": xt_np, "w1": w1_np, "w3": w3_np, "w2": w2_np})

    if trace:
        _ensure_ntff_hook()
    res = run_bass_kernel_spmd(nc, in_maps, core_ids=list(range(NCORES)),
                               trace=trace)

    out_full = np.zeros((T, H), dtype=np.float32)
    for c in range(NCORES):
        core_out = np.asarray(res.results[c]["out"]).astype(np.float32)
        ooff = 0
        for s, (qb, W, Wx) in enumerate(key):
            e, c0 = core_slot_job(c, s)
            n_e = int(bs[e])
            region = core_out[:, ooff:ooff + HC * W].reshape(TILE, HC, W)
            if n_e > 0:
                rows = region.transpose(2, 1, 0).reshape(W, H)[:n_e]
                out_full[starts[e]:starts[e] + n_e] += rows
            ooff += HC * W
    return out_full, res


def kernel(hiddens, w1_weight, w2_weight, w3_weight, batch_sizes):
    out, _ = _run(hiddens, w1_weight, w2_weight, w3_weight, batch_sizes)
    return out
